# revision 43
# baseline (speedup 1.0000x reference)
"""Trainium2 Bass kernel: RoPE causal attention block (B=2, S=2048, D=1024, H=16).

Sharding: (batch, head-group) across 8 cores — core c handles batch c//4 and
heads (c%4)*4 .. +4 (tensor-parallel qkv/o shards). Each core computes a
partial output projection over its 256 channels; host sums the 4 partials per
batch (the unshard step) and transposes back.

Device-side layout notes:
 - x is fed transposed (d on partitions) so it serves as both lhsT (v proj)
   and rhs (q/k proj) without on-chip transposes. x DMA is n-major (all 8
   k-blocks per 512-col chunk) so projection work starts ~3us in.
 - q/k are produced transposed [d_head, S] so scores come out as
   scores^T [k_pos, q_pos]; softmax denominators come from a ones column
   prepended to v (attn@v then yields den at PSUM row 0, z rows 1..64).
 - causal mask is applied on the PE: an extra accumulating matmul
   (maskq as lhsT, a head-broadcast identity as rhs) adds -1e9 above the
   diagonal, keeping DVE out of the scores->exp critical path.
 - exp/normalize: exp on ACT straight out of PSUM, reciprocal read directly
   from PSUM row 0 on DVE (1-instr approx), broadcast on GpSimd, multiplied
   during the PSUM->SBUF move on DVE.
 - ~4us of dummy matmuls at kernel start keep the PE HAM activity window
   busy during the input DMAs so the projection phase starts at 2.4 GHz.
All matmuls run in bf16 (1 cyc/row on PE vs 4 for f32).
"""

import sys

sys.path.insert(0, "/opt/trn_rl_repo")

import numpy as np
import ml_dtypes

import concourse.bass as bass  # noqa: F401
import concourse.tile as tile
from concourse import bacc, mybir
from concourse.bass_utils import run_bass_kernel_spmd

BF16 = mybir.dt.bfloat16
F32 = mybir.dt.float32

B, S, D = 2, 2048, 1024
H, DH = 16, 64
NCORE = 8
HPC = 4          # heads per core
KB = D // 128    # 8 k-blocks over the model dim
QC = 512         # q-chunk width
NQC = S // QC    # 4 q-chunks
NKT = S // 128   # 16 k-tiles
ROPE_BASE = 10000.0

_cache = {}


def _build():
    nc = bacc.Bacc("TRN2", target_bir_lowering=False, debug=False, num_devices=NCORE)

    x_d = nc.declare_dram_parameter("x", [128, 4, KB, 512], BF16, isOutput=False)
    wqk_d = nc.declare_dram_parameter("wqk", [128, 4, KB, 128], BF16, isOutput=False)
    wv_d = nc.declare_dram_parameter("wv", [128, KB, 256], BF16, isOutput=False)
    wo_d = nc.declare_dram_parameter("wo", [128, 2, 1024], BF16, isOutput=False)
    cos_d = nc.declare_dram_parameter("cos", [128, S], BF16, isOutput=False)
    sin_d = nc.declare_dram_parameter("sin", [128, S], BF16, isOutput=False)
    maskq_d = nc.declare_dram_parameter("maskq", [128, 128], BF16, isOutput=False)
    idm_d = nc.declare_dram_parameter("idm", [128, 2, 128], BF16, isOutput=False)
    out_d = nc.declare_dram_parameter("out", [D, S], BF16, isOutput=True)

    Exp = mybir.ActivationFunctionType.Exp

    with tile.TileContext(nc) as tc:
        with (
            tc.tile_pool(name="const", bufs=1) as cp,
            tc.tile_pool(name="ppool", bufs=2, space="PSUM") as ppool,
            tc.tile_pool(name="spool", bufs=2, space="PSUM") as spool,
            tc.tile_pool(name="zpool", bufs=2, space="PSUM") as zpool,
            tc.tile_pool(name="epool", bufs=18) as epool,
            tc.tile_pool(name="work", bufs=2) as work,
            tc.tile_pool(name="nrm", bufs=2) as nrm,
            tc.tile_pool(name="ob", bufs=8) as obp,
        ):
            # ---- input DMAs. The DMA engines round-robin across all live
            # transfers, so the j0-critical set (wv, wqk, x0, x1, cos, sin)
            # is issued immediately across the three rings, while everything
            # else (x2, x3, wo, maskq, idm) is gated behind x1's completion
            # via a 1-element touch op so it can't steal bandwidth early ----
            wv = cp.tile([128, KB, 256], BF16, tag="wv")
            nc.scalar.dma_start(wv[:], wv_d[:])
            # wqk mt-major: one 0.25MB DMA per head-group column, in the
            # order the q/k projection consumes them
            wqk = cp.tile([128, 4, KB, 128], BF16, tag="wqk")
            for mt in (0, 2, 1, 3):
                nc.scalar.dma_start(wqk[:, mt], wqk_d[:, mt])
            # x n-major as four separate tiles; the DRAM copy is laid out
            # n-major too, so each chunk half is one DMA with contiguous
            # 4KB-per-partition runs
            x_t = [cp.tile([128, KB, 512], BF16, tag=f"x{n}", name=f"x{n}") for n in range(4)]
            for n in range(2):
                nc.sync.dma_start(x_t[n][:, 0:4], x_d[:, n, 0:4])
                nc.gpsimd.dma_start(x_t[n][:, 4:8], x_d[:, n, 4:8])
            cos_sb = cp.tile([128, S], BF16, tag="cos")
            nc.sync.dma_start(cos_sb[:], cos_d[:])
            sin_sb = cp.tile([128, S], BF16, tag="sin")
            nc.gpsimd.dma_start(sin_sb[:], sin_d[:])

            # gate: a 1-elem copy reading x1 releases the non-critical DMAs
            maskq_sb = cp.tile([128, 128], BF16, tag="maskq")
            idm_sb = cp.tile([128, 2, 128], BF16, tag="idm")
            wo = cp.tile([128, 2, 1024], BF16, tag="wo")
            gate_src = x_t[1][0:1, 0, 0:1]
            nc.vector.tensor_copy(x_t[2][0:1, 0, 0:1], gate_src)
            nc.vector.tensor_copy(x_t[3][0:1, 0, 0:1], gate_src)
            nc.vector.tensor_copy(maskq_sb[0:1, 0:1], gate_src)
            nc.vector.tensor_copy(idm_sb[0:1, 0, 0:1], gate_src)
            nc.vector.tensor_copy(wo[0:1, 0, 0:1], gate_src)
            for n in range(2, 4):
                nc.sync.dma_start(x_t[n][:, 0:4], x_d[:, n, 0:4])
                nc.gpsimd.dma_start(x_t[n][:, 4:8], x_d[:, n, 4:8])
            nc.scalar.dma_start(maskq_sb[:], maskq_d[:])
            nc.scalar.dma_start(idm_sb[:], idm_d[:])
            nc.sync.dma_start(wo[:], wo_d[:])

            # tiny memsets first so the PE warm-up isn't gated behind the
            # big v_sb memset on the DVE queue
            wmm = work.tile([1, 64], BF16, tag="wmm")
            nc.vector.memset(wmm[:], 0.0)
            warm = work.tile([1, 8], F32, tag="warm")
            nc.vector.memset(warm[:], 0.0)
            # warm the ACT exp table while DMAs stream in
            nc.scalar.activation(warm[:], warm[:], Exp, scale=1.0)

            # ~2.5us of dummy matmuls: keep the PE HAM activity window busy
            # during the input DMAs so the proj phase starts warm (2.4 GHz)
            wps = ppool.tile([128, 512], F32, tag="proj", name="warmps")
            for _ in range(28):
                nc.tensor.matmul(wps[0:64, 0:64], wmm[:], wmm[:], start=True, stop=True)

            # v lhsT layout: col 0 = ones (softmax den lands in zt row 0),
            # cols 64..127 = v dims (z in zt rows 64..127 — PSUM partition
            # bases must be 32-aligned). Cols 1..63 are never read back but
            # must be finite for the matmul, so zero them.
            v_sb = cp.tile([128, NKT, HPC, 128], BF16, tag="v")
            nc.vector.memset(v_sb[:, :, :, 0:64], 0.0)
            nc.vector.memset(v_sb[:, :, :, 0:1], 1.0)

            # ---- v projection: v[s, c] (s on partitions), interleaved with
            # the q/k projection in x-DMA arrival order ----
            def emit_vproj(ts):
                for t in ts:
                    ps = ppool.tile([128, 512], F32, tag="proj", name=f"v{t}")
                    xt = x_t[t // 4]
                    c0 = (t % 4) * 128
                    for kb in range(KB):
                        nc.tensor.matmul(
                            ps[:, 0:256],
                            xt[:, kb, c0:c0 + 128],
                            wv[:, kb, :],
                            start=(kb == 0),
                            stop=(kb == KB - 1),
                        )
                    nc.vector.tensor_copy(
                        v_sb[:, t, :, 64:128],
                        ps[:, 0:256].rearrange("p (h d) -> p h d", h=HPC),
                    )

            # ---- q/k projection, transposed: [128 = 2 heads x 64, S] ----
            # mt 0,1 = q pairs; mt 2,3 = k pairs.
            T = [cp.tile([128, S], BF16, tag=f"T{mt}", name=f"T{mt}") for mt in range(4)]
            Tpre = [cp.tile([128, S], BF16, tag=f"Tpre{mt}", name=f"Tpre{mt}") for mt in range(4)]
            zsb = [cp.tile([128, S], BF16, tag=f"zsb{p}", name=f"zsb{p}") for p in range(2)]

            def emit_qk_chunk(n, mts, on_vector=False):
                # projection for one 512-col x chunk
                for mt in mts:
                    ps = ppool.tile([128, 512], F32, tag="proj", name=f"qk{mt}_{n}")
                    for kb in range(KB):
                        nc.tensor.matmul(
                            ps[:],
                            wqk[:, mt, kb, :],
                            x_t[n][:, kb],
                            start=(kb == 0),
                            stop=(kb == KB - 1),
                        )
                    dst = Tpre[mt][:, n * 512:(n + 1) * 512]
                    if on_vector:
                        nc.vector.tensor_copy(dst, ps[:])
                    else:
                        nc.scalar.copy(dst, ps[:])

            def emit_rope(nn, mts):
                # RoPE on one 1024-col half (partition-dim rotate-half via DMAs)
                lo = nn * 1024
                for mt in mts:
                    sw = work.tile([128, 1024], BF16, tag="sw")
                    for g in range(4):
                        src = (g + 1) if g % 2 == 0 else (g - 1)
                        nc.scalar.dma_start(
                            sw[g * 32:(g + 1) * 32, :],
                            Tpre[mt][src * 32:(src + 1) * 32, lo:lo + 1024],
                        )
                    t1 = work.tile([128, 1024], BF16, tag="t1")
                    nc.vector.tensor_mul(t1[:], Tpre[mt][:, lo:lo + 1024], cos_sb[:, lo:lo + 1024])
                    t2 = work.tile([128, 1024], BF16, tag="t2")
                    nc.vector.tensor_mul(t2[:], sw[:], sin_sb[:, lo:lo + 1024])
                    nc.vector.tensor_add(T[mt][:, lo:lo + 1024], t1[:], t2[:])

            def emit_qk_proj_half(nn, mts=(0, 2, 1, 3), on_vector=False):
                for mt in mts:
                    for n2 in range(2):
                        emit_qk_chunk(nn * 2 + n2, (mt,), on_vector)
                    emit_rope(nn, (mt,))

            # minimal front: only what attention j=0 needs (v tiles 0-3 and
            # the RoPE'd half-0 q/k). v t4-15 stream into the attention
            # groups below, where the exp stream paces the PE and leaves
            # slack to fill.
            emit_vproj(range(0, 4))
            emit_qk_chunk(0, (0, 2))
            emit_qk_chunk(1, (0, 2))
            emit_rope(0, (0, 2))

            # ---- attention (scores^T, exp, attn@v with den row) ----
            # second-half projections interleave at pair granularity: one
            # ~7us PE burst per attention group keeps ACT's 2-deep score
            # pipeline from draining during a single long projection stretch
            # half-1 q/k projection schedule: every mt must land strictly
            # before the first group whose SCORES read it (j=2 groups read
            # their own pair's half-1 q AND k) — emission is post-exp, so
            # an mt emitted at (j,p) is only safe for groups after (j,p)
            proj1_at = {(0, 1): (2,), (1, 0): (3,), (1, 1): (0,), (2, 0): (1,)}
            vproj_at = {(0, 1): range(4, 8), (1, 1): range(8, 12), (2, 1): range(12, 16)}
            for j in range(NQC):
                for pair in range(2):
                    kmax = 4 * (j + 1)
                    iorder = list(range(kmax))
                    exs = {}
                    for i in iorder:
                        qlo = max(0, 128 * i - QC * j)
                        diag = 128 * i >= QC * j
                        sc = spool.tile([128, 2, QC], F32, tag="sc")
                        for hh in range(2):
                            nc.tensor.matmul(
                                sc[:, hh, qlo:QC],
                                T[2 + pair][hh * 64:(hh + 1) * 64, 128 * i:128 * (i + 1)],
                                T[pair][hh * 64:(hh + 1) * 64, QC * j + qlo:QC * (j + 1)],
                                start=True,
                                stop=not diag,
                                skip_group_check=diag,
                            )
                        if diag:
                            # causal mask on the PE: accumulate maskq[q,k]
                            # (-1e9 above diagonal) over the 128-wide strip
                            nc.tensor.matmul(
                                sc[:, :, qlo:qlo + 128],
                                maskq_sb[:],
                                idm_sb[:],
                                start=False,
                                stop=True,
                                skip_group_check=True,
                            )
                        ex = epool.tile([128, 2, QC], BF16, tag="ex", name=f"ex{i}")
                        nc.scalar.activation(
                            ex[:, :, qlo:], sc[:, :, qlo:], Exp, scale=0.125
                        )
                        exs[i] = (ex, qlo)
                    # deferred projection / out-proj work lands here: after
                    # the group's scores+exp stream (so exp starts ASAP) but
                    # in the window where ACT paces the PE
                    if j == 0 and pair == 0:
                        # q/k half-0 for the second head pair (used from j0p1)
                        emit_qk_chunk(0, (1, 3))
                        emit_qk_chunk(1, (1, 3))
                        emit_rope(0, (1, 3))
                    if (j, pair) in proj1_at:
                        emit_qk_proj_half(1, mts=proj1_at[(j, pair)], on_vector=True)
                    if (j, pair) in vproj_at:
                        emit_vproj(vproj_at[(j, pair)])
                    if j > 0 and pair == 1:
                        _emit_outproj(nc, ppool, obp, wo, zsb, out_d, j - 1)
                    for hh in range(2):
                        zt = zpool.tile([128, QC], F32, tag="zt")
                        for n_i, i in enumerate(iorder):
                            ex, qlo = exs[i]
                            nc.tensor.matmul(
                                zt[:, qlo:],
                                v_sb[:, i, pair * 2 + hh, :],
                                ex[:, hh, qlo:],
                                start=(n_i == 0),
                                stop=(n_i == kmax - 1),
                            )
                        # normalize: z rows 64..127 by den row 0
                        # (copy den off PSUM first: the custom-DVE recip's
                        # PSUM read path races the PE accumulation)
                        den = nrm.tile([1, QC], F32, tag="den")
                        nc.vector.tensor_copy(den[:], zt[0:1, :])
                        rec = nrm.tile([1, QC], F32, tag="rec")
                        nc.vector.reciprocal_approx_fast(rec[:], den[:])
                        bc = nrm.tile([64, QC], F32, tag="bc")
                        nc.gpsimd.partition_broadcast(bc[:], rec[:])
                        nc.vector.tensor_mul(
                            zsb[pair][hh * 64:(hh + 1) * 64, QC * j:QC * (j + 1)],
                            zt[64:128, :],
                            bc[:],
                        )
            # final out-proj: attention pools are idle now — rotate po tiles
            # through 6 PSUM slots so the MMs aren't paced by the ob copies
            _emit_outproj(nc, ppool, obp, wo, zsb, out_d, NQC - 1,
                          psum_pools=[(ppool, "proj"), (spool, "sc"), (zpool, "zt")])

    nc.compile()
    return nc


def _emit_outproj(nc, ppool, obp, wo, zsb, out_d, j, pairs=(0, 1), pos=None, done=True,
                  psum_pools=None):
    pools = psum_pools or [(ppool, "proj")]
    for m in range(8):
        pool, ptag = pools[m % len(pools)]
        po = pool.tile([128, 512], F32, tag=ptag,
                       name=f"po{j}_{m}") if pos is None else pos[m]
        for p in pairs:
            nc.tensor.matmul(
                po[:],
                wo[:, p, m * 128:(m + 1) * 128],
                zsb[p][:, QC * j:QC * (j + 1)],
                start=(p == pairs[0] and pos is None),
                stop=(p == pairs[-1] and done),
            )
        if done:
            # final chunk: ACT and gpsimd are idle — split copies and DMAs
            # across engines/queues. Mid-attention they'd steal time from
            # exp (ACT) and the norm broadcasts (gpsimd), so DVE/sync only.
            final = psum_pools is not None
            ob = obp.tile([128, 512], BF16, tag="ob", name=f"ob{j}_{m}")
            if final and m % 2 == 0:
                nc.scalar.copy(ob[:], po[:])
            else:
                nc.vector.tensor_copy(ob[:], po[:])
            dq = nc.gpsimd if final and m % 2 == 1 else nc.sync
            dq.dma_start(out_d[m * 128:(m + 1) * 128, QC * j:QC * (j + 1)], ob[:])


def _rope_tables():
    inv_freq = 1.0 / (ROPE_BASE ** (np.arange(0, DH, 2, dtype=np.float32) / DH))
    t = np.arange(S, dtype=np.float32)
    freqs = np.outer(t, inv_freq)            # [S, 32]
    cosT = np.cos(freqs).T                   # [32, S]
    sinT = np.sin(freqs).T
    cos128 = np.concatenate([cosT, cosT, cosT, cosT], axis=0)
    sin128 = np.concatenate([-sinT, sinT, -sinT, sinT], axis=0)
    return cos128.astype(ml_dtypes.bfloat16), sin128.astype(ml_dtypes.bfloat16)


def _prep_in_maps(x, w_qkv, w_o):
    cos128, sin128 = _rope_tables()
    kp, qc = np.meshgrid(np.arange(128), np.arange(128), indexing="ij")
    # maskq[q, k] = -1e9 where k > q (lhsT of the PE mask matmul)
    maskq = np.where(kp > qc, -1e9, 0.0).T.astype(np.float32)
    idm1 = np.eye(128, dtype=np.float32)                              # [p, q]
    idm = np.ascontiguousarray(np.stack([idm1, idm1], axis=1))        # [128, 2, 128]

    in_maps = []
    for c in range(NCORE):
        b, hb = c // 4, (c % 4) * HPC
        xb = np.ascontiguousarray(x[b].T)                        # [D, S]
        x_sb = xb.reshape(KB, 128, S).transpose(1, 0, 2)         # [128, KB, S]
        x_sb = x_sb.reshape(128, KB, 4, 512).transpose(0, 2, 1, 3)  # [128, 4, KB, 512]

        wqk = np.empty((128, 4, KB, 128), np.float32)
        for pair in range(2):
            qrows = w_qkv[(hb + 2 * pair) * DH:(hb + 2 * pair + 2) * DH, :]   # [128, D]
            krows = w_qkv[D + (hb + 2 * pair) * DH:D + (hb + 2 * pair + 2) * DH, :]
            wqk[:, pair] = qrows.T.reshape(KB, 128, 128).transpose(1, 0, 2)
            wqk[:, 2 + pair] = krows.T.reshape(KB, 128, 128).transpose(1, 0, 2)

        vrows = w_qkv[2 * D + hb * DH:2 * D + (hb + HPC) * DH, :]             # [256, D]
        wv = vrows.T.reshape(KB, 128, 256).transpose(1, 0, 2)                 # [128, KB, 256]

        wo_blk = w_o[:, hb * DH:hb * DH + 256]                                # [1024, 256]
        wo = wo_blk.T.reshape(2, 128, 1024).transpose(1, 0, 2)                # [128, 2, 1024]

        in_maps.append({
            "x": x_sb.astype(ml_dtypes.bfloat16),
            "wqk": wqk.astype(ml_dtypes.bfloat16),
            "wv": wv.astype(ml_dtypes.bfloat16),
            "wo": wo.astype(ml_dtypes.bfloat16),
            "cos": cos128,
            "sin": sin128,
            "maskq": maskq.astype(ml_dtypes.bfloat16),
            "idm": idm.astype(ml_dtypes.bfloat16),
        })
    return in_maps


def get_nc():
    if "nc" not in _cache:
        _cache["nc"] = _build()
    return _cache["nc"]


def run(x, w_qkv, w_o, **runkw):
    nc = get_nc()
    in_maps = _prep_in_maps(np.asarray(x), np.asarray(w_qkv), np.asarray(w_o))
    res = run_bass_kernel_spmd(nc, in_maps, core_ids=list(range(NCORE)), **runkw)
    out = np.zeros((B, S, D), np.float32)
    for c in range(NCORE):
        out[c // 4] += res.results[c]["out"].astype(np.float32).T
    return out, res


def kernel(x, w_qkv, w_o):
    out, _ = run(x, w_qkv, w_o)
    return out


# revision 47
# speedup vs baseline: 1.0527x; 1.0527x over previous
"""Trainium2 Bass kernel: RoPE causal attention block (B=2, S=2048, D=1024, H=16).

Sharding: (batch, head-group) across 8 cores — core c handles batch c//4 and
heads (c%4)*4 .. +4 (tensor-parallel qkv/o shards). Each core computes a
partial output projection over its 256 channels; host sums the 4 partials per
batch (the unshard step) and transposes back.

Device-side layout notes:
 - x is fed transposed (d on partitions) so it serves as both lhsT (v proj)
   and rhs (q/k proj) without on-chip transposes. x DMA is n-major (all 8
   k-blocks per 512-col chunk) so projection work starts ~3us in.
 - q/k are produced transposed [d_head, S] so scores come out as
   scores^T [k_pos, q_pos]; softmax denominators come from a ones column
   prepended to v (attn@v then yields den at PSUM row 0, z rows 1..64).
 - causal mask is applied on the PE: an extra accumulating matmul
   (maskq as lhsT, a head-broadcast identity as rhs) adds -1e9 above the
   diagonal, keeping DVE out of the scores->exp critical path.
 - exp/normalize: exp on ACT straight out of PSUM, reciprocal read directly
   from PSUM row 0 on DVE (1-instr approx), broadcast on GpSimd, multiplied
   during the PSUM->SBUF move on DVE.
 - ~4us of dummy matmuls at kernel start keep the PE HAM activity window
   busy during the input DMAs so the projection phase starts at 2.4 GHz.
All matmuls run in bf16 (1 cyc/row on PE vs 4 for f32).
"""

import sys

sys.path.insert(0, "/opt/trn_rl_repo")

import numpy as np
import ml_dtypes

import concourse.bass as bass  # noqa: F401
import concourse.tile as tile
from concourse import bacc, mybir
from concourse.bass_utils import run_bass_kernel_spmd

BF16 = mybir.dt.bfloat16
F32 = mybir.dt.float32

B, S, D = 2, 2048, 1024
H, DH = 16, 64
NCORE = 8
HPC = 4          # heads per core
KB = D // 128    # 8 k-blocks over the model dim
QC = 512         # q-chunk width
NQC = S // QC    # 4 q-chunks
NKT = S // 128   # 16 k-tiles
ROPE_BASE = 10000.0

_cache = {}


def _build():
    nc = bacc.Bacc("TRN2", target_bir_lowering=False, debug=False, num_devices=NCORE)

    x_d = nc.declare_dram_parameter("x", [128, 4, KB, 512], BF16, isOutput=False)
    wqk_d = nc.declare_dram_parameter("wqk", [128, 4, KB, 128], BF16, isOutput=False)
    wv_d = nc.declare_dram_parameter("wv", [128, KB, 256], BF16, isOutput=False)
    wo_d = nc.declare_dram_parameter("wo", [128, 2, 1024], BF16, isOutput=False)
    cos_d = nc.declare_dram_parameter("cos", [128, S], BF16, isOutput=False)
    sin_d = nc.declare_dram_parameter("sin", [128, S], BF16, isOutput=False)
    maskq_d = nc.declare_dram_parameter("maskq", [128, 128], BF16, isOutput=False)
    idm_d = nc.declare_dram_parameter("idm", [128, 2, 128], BF16, isOutput=False)
    out_d = nc.declare_dram_parameter("out", [D, S], BF16, isOutput=True)

    Exp = mybir.ActivationFunctionType.Exp

    with tile.TileContext(nc) as tc:
        with (
            tc.tile_pool(name="const", bufs=1) as cp,
            tc.tile_pool(name="ppool", bufs=2, space="PSUM") as ppool,
            tc.tile_pool(name="spool", bufs=2, space="PSUM") as spool,
            tc.tile_pool(name="zpool", bufs=2, space="PSUM") as zpool,
            tc.tile_pool(name="epool", bufs=18) as epool,
            tc.tile_pool(name="work", bufs=2) as work,
            tc.tile_pool(name="nrm", bufs=2) as nrm,
            tc.tile_pool(name="ob", bufs=8) as obp,
        ):
            # ---- input DMAs. The DMA engines round-robin across all live
            # transfers, so the j0-critical set (wv, wqk, x0, x1, cos, sin)
            # is issued immediately across the three rings, while everything
            # else (x2, x3, wo, maskq, idm) is gated behind x1's completion
            # via a 1-element touch op so it can't steal bandwidth early ----
            wv = cp.tile([128, KB, 256], BF16, tag="wv")
            nc.scalar.dma_start(wv[:], wv_d[:])
            # wqk mt-major: one 0.25MB DMA per head-group column, in the
            # order the q/k projection consumes them
            wqk = cp.tile([128, 4, KB, 128], BF16, tag="wqk")
            for mt in (0, 2, 1, 3):
                nc.scalar.dma_start(wqk[:, mt], wqk_d[:, mt])
            # x n-major as four separate tiles; the DRAM copy is laid out
            # n-major too, so each chunk half is one DMA with contiguous
            # 4KB-per-partition runs
            x_t = [cp.tile([128, KB, 512], BF16, tag=f"x{n}", name=f"x{n}") for n in range(4)]
            for n in range(2):
                nc.sync.dma_start(x_t[n][:, 0:4], x_d[:, n, 0:4])
                nc.gpsimd.dma_start(x_t[n][:, 4:8], x_d[:, n, 4:8])
            cos_sb = cp.tile([128, S], BF16, tag="cos")
            nc.sync.dma_start(cos_sb[:], cos_d[:])
            sin_sb = cp.tile([128, S], BF16, tag="sin")
            nc.gpsimd.dma_start(sin_sb[:], sin_d[:])

            # gate: a 1-elem copy reading x1 releases the non-critical DMAs
            maskq_sb = cp.tile([128, 128], BF16, tag="maskq")
            idm_sb = cp.tile([128, 2, 128], BF16, tag="idm")
            wo = cp.tile([128, 2, 1024], BF16, tag="wo")
            gate_src = x_t[1][0:1, 0, 0:1]
            nc.vector.tensor_copy(x_t[2][0:1, 0, 0:1], gate_src)
            nc.vector.tensor_copy(x_t[3][0:1, 0, 0:1], gate_src)
            nc.vector.tensor_copy(maskq_sb[0:1, 0:1], gate_src)
            nc.vector.tensor_copy(idm_sb[0:1, 0, 0:1], gate_src)
            nc.vector.tensor_copy(wo[0:1, 0, 0:1], gate_src)
            for n in range(2, 4):
                nc.sync.dma_start(x_t[n][:, 0:4], x_d[:, n, 0:4])
                nc.gpsimd.dma_start(x_t[n][:, 4:8], x_d[:, n, 4:8])
            nc.scalar.dma_start(maskq_sb[:], maskq_d[:])
            nc.scalar.dma_start(idm_sb[:], idm_d[:])
            nc.sync.dma_start(wo[:], wo_d[:])

            # tiny memsets first so the PE warm-up isn't gated behind the
            # big v_sb memset on the DVE queue
            wmm = work.tile([1, 64], BF16, tag="wmm")
            nc.vector.memset(wmm[:], 0.0)
            warm = work.tile([1, 8], F32, tag="warm")
            nc.vector.memset(warm[:], 0.0)
            # warm the ACT exp table while DMAs stream in
            nc.scalar.activation(warm[:], warm[:], Exp, scale=1.0)

            # ~2.5us of dummy matmuls: keep the PE HAM activity window busy
            # during the input DMAs so the proj phase starts warm (2.4 GHz)
            wps = ppool.tile([128, 512], F32, tag="proj", name="warmps")
            for _ in range(28):
                nc.tensor.matmul(wps[0:64, 0:64], wmm[:], wmm[:], start=True, stop=True)

            # v lhsT layout: col 0 = ones (softmax den lands in zt row 0),
            # cols 64..127 = v dims (z in zt rows 64..127 — PSUM partition
            # bases must be 32-aligned). Cols 1..63 are never read back but
            # must be finite for the matmul, so zero them.
            v_sb = cp.tile([128, NKT, HPC, 128], BF16, tag="v")
            nc.vector.memset(v_sb[:, :, :, 0:64], 0.0)
            nc.vector.memset(v_sb[:, :, :, 0:1], 1.0)

            # ---- v projection: v[s, c] (s on partitions), interleaved with
            # the q/k projection in x-DMA arrival order ----
            def emit_vproj(ts):
                for t in ts:
                    ps = ppool.tile([128, 512], F32, tag="proj", name=f"v{t}")
                    xt = x_t[t // 4]
                    c0 = (t % 4) * 128
                    for kb in range(KB):
                        nc.tensor.matmul(
                            ps[:, 0:256],
                            xt[:, kb, c0:c0 + 128],
                            wv[:, kb, :],
                            start=(kb == 0),
                            stop=(kb == KB - 1),
                        )
                    nc.vector.tensor_copy(
                        v_sb[:, t, :, 64:128],
                        ps[:, 0:256].rearrange("p (h d) -> p h d", h=HPC),
                    )

            # ---- q/k projection, transposed: [128 = 2 heads x 64, S] ----
            # mt 0,1 = q pairs; mt 2,3 = k pairs.
            T = [cp.tile([128, S], BF16, tag=f"T{mt}", name=f"T{mt}") for mt in range(4)]
            Tpre = [cp.tile([128, S], BF16, tag=f"Tpre{mt}", name=f"Tpre{mt}") for mt in range(4)]
            zsb = [cp.tile([128, S], BF16, tag=f"zsb{p}", name=f"zsb{p}") for p in range(2)]

            def emit_qk_chunk(n, mts, on_vector=False):
                # projection for one 512-col x chunk
                for mt in mts:
                    ps = ppool.tile([128, 512], F32, tag="proj", name=f"qk{mt}_{n}")
                    for kb in range(KB):
                        nc.tensor.matmul(
                            ps[:],
                            wqk[:, mt, kb, :],
                            x_t[n][:, kb],
                            start=(kb == 0),
                            stop=(kb == KB - 1),
                        )
                    dst = Tpre[mt][:, n * 512:(n + 1) * 512]
                    if on_vector:
                        nc.vector.tensor_copy(dst, ps[:])
                    else:
                        nc.scalar.copy(dst, ps[:])

            def emit_rope(nn, mts):
                # RoPE on one 1024-col half (partition-dim rotate-half via DMAs)
                lo = nn * 1024
                for mt in mts:
                    sw = work.tile([128, 1024], BF16, tag="sw")
                    for g in range(4):
                        src = (g + 1) if g % 2 == 0 else (g - 1)
                        # sync queue: a DMA issue occupies the issuing engine
                        # ~0.65us, and the scalar engine is the exp pacer
                        # during the attention groups where half-1 RoPE runs
                        nc.sync.dma_start(
                            sw[g * 32:(g + 1) * 32, :],
                            Tpre[mt][src * 32:(src + 1) * 32, lo:lo + 1024],
                        )
                    t1 = work.tile([128, 1024], BF16, tag="t1")
                    nc.vector.tensor_mul(t1[:], Tpre[mt][:, lo:lo + 1024], cos_sb[:, lo:lo + 1024])
                    t2 = work.tile([128, 1024], BF16, tag="t2")
                    nc.vector.tensor_mul(t2[:], sw[:], sin_sb[:, lo:lo + 1024])
                    nc.vector.tensor_add(T[mt][:, lo:lo + 1024], t1[:], t2[:])

            def emit_qk_proj_half(nn, mts=(0, 2, 1, 3), on_vector=False):
                for mt in mts:
                    for n2 in range(2):
                        emit_qk_chunk(nn * 2 + n2, (mt,), on_vector)
                    emit_rope(nn, (mt,))

            # minimal front: only what attention j=0 needs (v tiles 0-3 and
            # the RoPE'd half-0 q/k). v t4-15 stream into the attention
            # groups below, where the exp stream paces the PE and leaves
            # slack to fill.
            emit_vproj(range(0, 4))
            emit_qk_chunk(0, (0, 2))
            emit_qk_chunk(1, (0, 2))
            emit_rope(0, (0, 2))

            # ---- attention (scores^T, exp, attn@v with den row) ----
            # second-half projections interleave at pair granularity: one
            # ~7us PE burst per attention group keeps ACT's 2-deep score
            # pipeline from draining during a single long projection stretch
            # half-1 q/k projection schedule: every mt must land strictly
            # before the first group whose SCORES read it (j=2 groups read
            # their own pair's half-1 q AND k) — emission is post-exp, so
            # an mt emitted at (j,p) is only safe for groups after (j,p)
            proj1_at = {(0, 1): (2,), (1, 0): (3,), (1, 1): (0,), (2, 0): (1,)}
            vproj_at = {(0, 1): range(4, 8), (1, 1): range(8, 12), (2, 1): range(12, 16)}
            for j in range(NQC):
                for pair in range(2):
                    kmax = 4 * (j + 1)
                    iorder = list(range(kmax))
                    exs = {}
                    for i in iorder:
                        qlo = max(0, 128 * i - QC * j)
                        diag = 128 * i >= QC * j
                        sc = spool.tile([128, 2, QC], F32, tag="sc")
                        for hh in range(2):
                            nc.tensor.matmul(
                                sc[:, hh, qlo:QC],
                                T[2 + pair][hh * 64:(hh + 1) * 64, 128 * i:128 * (i + 1)],
                                T[pair][hh * 64:(hh + 1) * 64, QC * j + qlo:QC * (j + 1)],
                                start=True,
                                stop=not diag,
                                skip_group_check=diag,
                            )
                        if diag:
                            # causal mask on the PE: accumulate maskq[q,k]
                            # (-1e9 above diagonal) over the 128-wide strip
                            nc.tensor.matmul(
                                sc[:, :, qlo:qlo + 128],
                                maskq_sb[:],
                                idm_sb[:],
                                start=False,
                                stop=True,
                                skip_group_check=True,
                            )
                        ex = epool.tile([128, 2, QC], BF16, tag="ex", name=f"ex{i}")
                        nc.scalar.activation(
                            ex[:, :, qlo:], sc[:, :, qlo:], Exp, scale=0.125
                        )
                        exs[i] = (ex, qlo)
                    # deferred projection / out-proj work lands here: after
                    # the group's scores+exp stream (so exp starts ASAP) but
                    # in the window where ACT paces the PE
                    if j == 0 and pair == 0:
                        # q/k half-0 for the second head pair (used from j0p1)
                        emit_qk_chunk(0, (1, 3))
                        emit_qk_chunk(1, (1, 3))
                        emit_rope(0, (1, 3))
                    if (j, pair) in proj1_at:
                        emit_qk_proj_half(1, mts=proj1_at[(j, pair)], on_vector=True)
                    if (j, pair) in vproj_at:
                        emit_vproj(vproj_at[(j, pair)])
                    if j > 0:
                        # previous chunk's out-proj, half per group for
                        # finer-grained PE fill
                        _emit_outproj(nc, ppool, obp, wo, zsb, out_d, j - 1,
                                      ms=range(4) if pair == 0 else range(4, 8))
                    for hh in range(2):
                        zt = zpool.tile([128, QC], F32, tag="zt")
                        for n_i, i in enumerate(iorder):
                            ex, qlo = exs[i]
                            nc.tensor.matmul(
                                zt[:, qlo:],
                                v_sb[:, i, pair * 2 + hh, :],
                                ex[:, hh, qlo:],
                                start=(n_i == 0),
                                stop=(n_i == kmax - 1),
                            )
                        # normalize: z rows 64..127 by den row 0
                        # (copy den off PSUM first: the custom-DVE recip's
                        # PSUM read path races the PE accumulation)
                        den = nrm.tile([1, QC], F32, tag="den")
                        nc.vector.tensor_copy(den[:], zt[0:1, :])
                        rec = nrm.tile([1, QC], F32, tag="rec")
                        nc.vector.reciprocal_approx_fast(rec[:], den[:])
                        bc = nrm.tile([64, QC], F32, tag="bc")
                        nc.gpsimd.partition_broadcast(bc[:], rec[:])
                        nc.vector.tensor_mul(
                            zsb[pair][hh * 64:(hh + 1) * 64, QC * j:QC * (j + 1)],
                            zt[64:128, :],
                            bc[:],
                        )
            # final out-proj: attention pools are idle now — rotate po tiles
            # through 6 PSUM slots so the MMs aren't paced by the ob copies
            _emit_outproj(nc, ppool, obp, wo, zsb, out_d, NQC - 1,
                          psum_pools=[(ppool, "proj"), (spool, "sc"), (zpool, "zt")])

    nc.compile()
    return nc


def _emit_outproj(nc, ppool, obp, wo, zsb, out_d, j, pairs=(0, 1), pos=None, done=True,
                  psum_pools=None, ms=range(8)):
    pools = psum_pools or [(ppool, "proj")]
    for m in ms:
        pool, ptag = pools[m % len(pools)]
        po = pool.tile([128, 512], F32, tag=ptag,
                       name=f"po{j}_{m}") if pos is None else pos[m]
        for p in pairs:
            nc.tensor.matmul(
                po[:],
                wo[:, p, m * 128:(m + 1) * 128],
                zsb[p][:, QC * j:QC * (j + 1)],
                start=(p == pairs[0] and pos is None),
                stop=(p == pairs[-1] and done),
            )
        if done:
            # final chunk: ACT and gpsimd are idle — split copies and DMAs
            # across engines/queues. Mid-attention they'd steal time from
            # exp (ACT) and the norm broadcasts (gpsimd), so DVE/sync only.
            final = psum_pools is not None
            ob = obp.tile([128, 512], BF16, tag="ob", name=f"ob{j}_{m}")
            if final and m % 2 == 0:
                nc.scalar.copy(ob[:], po[:])
            else:
                nc.vector.tensor_copy(ob[:], po[:])
            dq = nc.gpsimd if final and m % 2 == 1 else nc.sync
            dq.dma_start(out_d[m * 128:(m + 1) * 128, QC * j:QC * (j + 1)], ob[:])


def _rope_tables():
    inv_freq = 1.0 / (ROPE_BASE ** (np.arange(0, DH, 2, dtype=np.float32) / DH))
    t = np.arange(S, dtype=np.float32)
    freqs = np.outer(t, inv_freq)            # [S, 32]
    cosT = np.cos(freqs).T                   # [32, S]
    sinT = np.sin(freqs).T
    cos128 = np.concatenate([cosT, cosT, cosT, cosT], axis=0)
    sin128 = np.concatenate([-sinT, sinT, -sinT, sinT], axis=0)
    return cos128.astype(ml_dtypes.bfloat16), sin128.astype(ml_dtypes.bfloat16)


def _prep_in_maps(x, w_qkv, w_o):
    cos128, sin128 = _rope_tables()
    kp, qc = np.meshgrid(np.arange(128), np.arange(128), indexing="ij")
    # maskq[q, k] = -1e9 where k > q (lhsT of the PE mask matmul)
    maskq = np.where(kp > qc, -1e9, 0.0).T.astype(np.float32)
    idm1 = np.eye(128, dtype=np.float32)                              # [p, q]
    idm = np.ascontiguousarray(np.stack([idm1, idm1], axis=1))        # [128, 2, 128]

    in_maps = []
    for c in range(NCORE):
        b, hb = c // 4, (c % 4) * HPC
        xb = np.ascontiguousarray(x[b].T)                        # [D, S]
        x_sb = xb.reshape(KB, 128, S).transpose(1, 0, 2)         # [128, KB, S]
        x_sb = x_sb.reshape(128, KB, 4, 512).transpose(0, 2, 1, 3)  # [128, 4, KB, 512]

        wqk = np.empty((128, 4, KB, 128), np.float32)
        for pair in range(2):
            qrows = w_qkv[(hb + 2 * pair) * DH:(hb + 2 * pair + 2) * DH, :]   # [128, D]
            krows = w_qkv[D + (hb + 2 * pair) * DH:D + (hb + 2 * pair + 2) * DH, :]
            wqk[:, pair] = qrows.T.reshape(KB, 128, 128).transpose(1, 0, 2)
            wqk[:, 2 + pair] = krows.T.reshape(KB, 128, 128).transpose(1, 0, 2)

        vrows = w_qkv[2 * D + hb * DH:2 * D + (hb + HPC) * DH, :]             # [256, D]
        wv = vrows.T.reshape(KB, 128, 256).transpose(1, 0, 2)                 # [128, KB, 256]

        wo_blk = w_o[:, hb * DH:hb * DH + 256]                                # [1024, 256]
        wo = wo_blk.T.reshape(2, 128, 1024).transpose(1, 0, 2)                # [128, 2, 1024]

        in_maps.append({
            "x": x_sb.astype(ml_dtypes.bfloat16),
            "wqk": wqk.astype(ml_dtypes.bfloat16),
            "wv": wv.astype(ml_dtypes.bfloat16),
            "wo": wo.astype(ml_dtypes.bfloat16),
            "cos": cos128,
            "sin": sin128,
            "maskq": maskq.astype(ml_dtypes.bfloat16),
            "idm": idm.astype(ml_dtypes.bfloat16),
        })
    return in_maps


def get_nc():
    if "nc" not in _cache:
        _cache["nc"] = _build()
    return _cache["nc"]


def run(x, w_qkv, w_o, **runkw):
    nc = get_nc()
    in_maps = _prep_in_maps(np.asarray(x), np.asarray(w_qkv), np.asarray(w_o))
    res = run_bass_kernel_spmd(nc, in_maps, core_ids=list(range(NCORE)), **runkw)
    out = np.zeros((B, S, D), np.float32)
    for c in range(NCORE):
        out[c // 4] += res.results[c]["out"].astype(np.float32).T
    return out, res


def kernel(x, w_qkv, w_o):
    out, _ = run(x, w_qkv, w_o)
    return out


# revision 49
# speedup vs baseline: 1.0547x; 1.0019x over previous
"""Trainium2 Bass kernel: RoPE causal attention block (B=2, S=2048, D=1024, H=16).

Sharding: (batch, head-group) across 8 cores — core c handles batch c//4 and
heads (c%4)*4 .. +4 (tensor-parallel qkv/o shards). Each core computes a
partial output projection over its 256 channels; host sums the 4 partials per
batch (the unshard step) and transposes back.

Device-side layout notes:
 - x is fed transposed (d on partitions) so it serves as both lhsT (v proj)
   and rhs (q/k proj) without on-chip transposes. x DMA is n-major (all 8
   k-blocks per 512-col chunk) so projection work starts ~3us in.
 - q/k are produced transposed [d_head, S] so scores come out as
   scores^T [k_pos, q_pos]; softmax denominators come from a ones column
   prepended to v (attn@v then yields den at PSUM row 0, z rows 1..64).
 - causal mask is applied on the PE: an extra accumulating matmul
   (maskq as lhsT, a head-broadcast identity as rhs) adds -1e9 above the
   diagonal, keeping DVE out of the scores->exp critical path.
 - exp/normalize: exp on ACT straight out of PSUM, reciprocal read directly
   from PSUM row 0 on DVE (1-instr approx), broadcast on GpSimd, multiplied
   during the PSUM->SBUF move on DVE.
 - ~4us of dummy matmuls at kernel start keep the PE HAM activity window
   busy during the input DMAs so the projection phase starts at 2.4 GHz.
All matmuls run in bf16 (1 cyc/row on PE vs 4 for f32).
"""

import sys

sys.path.insert(0, "/opt/trn_rl_repo")

import numpy as np
import ml_dtypes

import concourse.bass as bass  # noqa: F401
import concourse.tile as tile
from concourse import bacc, mybir
from concourse.bass_utils import run_bass_kernel_spmd

BF16 = mybir.dt.bfloat16
F32 = mybir.dt.float32

B, S, D = 2, 2048, 1024
H, DH = 16, 64
NCORE = 8
HPC = 4          # heads per core
KB = D // 128    # 8 k-blocks over the model dim
QC = 512         # q-chunk width
NQC = S // QC    # 4 q-chunks
NKT = S // 128   # 16 k-tiles
ROPE_BASE = 10000.0

_cache = {}


def _build():
    nc = bacc.Bacc("TRN2", target_bir_lowering=False, debug=False, num_devices=NCORE)

    x_d = nc.declare_dram_parameter("x", [128, 4, KB, 512], BF16, isOutput=False)
    wqk_d = nc.declare_dram_parameter("wqk", [128, 4, KB, 128], BF16, isOutput=False)
    wv_d = nc.declare_dram_parameter("wv", [128, KB, 256], BF16, isOutput=False)
    wo_d = nc.declare_dram_parameter("wo", [128, 2, 1024], BF16, isOutput=False)
    cos_d = nc.declare_dram_parameter("cos", [128, S], BF16, isOutput=False)
    sin_d = nc.declare_dram_parameter("sin", [128, S], BF16, isOutput=False)
    maskq_d = nc.declare_dram_parameter("maskq", [128, 128], BF16, isOutput=False)
    idm_d = nc.declare_dram_parameter("idm", [128, 2, 128], BF16, isOutput=False)
    out_d = nc.declare_dram_parameter("out", [D, S], BF16, isOutput=True)

    Exp = mybir.ActivationFunctionType.Exp

    with tile.TileContext(nc) as tc:
        with (
            tc.tile_pool(name="const", bufs=1) as cp,
            tc.tile_pool(name="ppool", bufs=2, space="PSUM") as ppool,
            tc.tile_pool(name="spool", bufs=2, space="PSUM") as spool,
            tc.tile_pool(name="zpool", bufs=2, space="PSUM") as zpool,
            tc.tile_pool(name="epool", bufs=18) as epool,
            tc.tile_pool(name="work", bufs=2) as work,
            tc.tile_pool(name="nrm", bufs=2) as nrm,
            tc.tile_pool(name="ob", bufs=8) as obp,
        ):
            # ---- input DMAs. The DMA engines round-robin across all live
            # transfers, so the j0-critical set (wv, wqk, x0, x1, cos, sin)
            # is issued immediately across the three rings, while everything
            # else (x2, x3, wo, maskq, idm) is gated behind x1's completion
            # via a 1-element touch op so it can't steal bandwidth early ----
            wv = cp.tile([128, KB, 256], BF16, tag="wv")
            nc.scalar.dma_start(wv[:], wv_d[:])
            # wqk mt-major: one 0.25MB DMA per head-group column, in the
            # order the q/k projection consumes them
            wqk = cp.tile([128, 4, KB, 128], BF16, tag="wqk")
            for mt in (0, 2, 1, 3):
                nc.scalar.dma_start(wqk[:, mt], wqk_d[:, mt])
            # x n-major as four separate tiles; the DRAM copy is laid out
            # n-major too, so each chunk half is one DMA with contiguous
            # 4KB-per-partition runs
            x_t = [cp.tile([128, KB, 512], BF16, tag=f"x{n}", name=f"x{n}") for n in range(4)]
            for n in range(2):
                nc.sync.dma_start(x_t[n][:, 0:4], x_d[:, n, 0:4])
                nc.gpsimd.dma_start(x_t[n][:, 4:8], x_d[:, n, 4:8])
            cos_sb = cp.tile([128, S], BF16, tag="cos")
            nc.sync.dma_start(cos_sb[:], cos_d[:])
            sin_sb = cp.tile([128, S], BF16, tag="sin")
            nc.gpsimd.dma_start(sin_sb[:], sin_d[:])

            # gate: a 1-elem copy reading x1 releases the non-critical DMAs
            maskq_sb = cp.tile([128, 128], BF16, tag="maskq")
            idm_sb = cp.tile([128, 2, 128], BF16, tag="idm")
            wo = cp.tile([128, 2, 1024], BF16, tag="wo")
            gate_src = x_t[1][0:1, 0, 0:1]
            nc.vector.tensor_copy(x_t[2][0:1, 0, 0:1], gate_src)
            nc.vector.tensor_copy(x_t[3][0:1, 0, 0:1], gate_src)
            nc.vector.tensor_copy(maskq_sb[0:1, 0:1], gate_src)
            nc.vector.tensor_copy(idm_sb[0:1, 0, 0:1], gate_src)
            nc.vector.tensor_copy(wo[0:1, 0, 0:1], gate_src)
            for n in range(2, 4):
                nc.sync.dma_start(x_t[n][:, 0:4], x_d[:, n, 0:4])
                nc.gpsimd.dma_start(x_t[n][:, 4:8], x_d[:, n, 4:8])
            nc.scalar.dma_start(maskq_sb[:], maskq_d[:])
            nc.scalar.dma_start(idm_sb[:], idm_d[:])
            nc.sync.dma_start(wo[:], wo_d[:])

            # tiny memsets first so the PE warm-up isn't gated behind the
            # big v_sb memset on the DVE queue
            wmm = work.tile([1, 64], BF16, tag="wmm")
            nc.vector.memset(wmm[:], 0.0)
            warm = work.tile([1, 8], F32, tag="warm")
            nc.vector.memset(warm[:], 0.0)
            # warm the ACT exp table while DMAs stream in
            nc.scalar.activation(warm[:], warm[:], Exp, scale=1.0)

            # ~2.5us of dummy matmuls: keep the PE HAM activity window busy
            # during the input DMAs so the proj phase starts warm (2.4 GHz)
            wps = ppool.tile([128, 512], F32, tag="proj", name="warmps")
            for _ in range(28):
                nc.tensor.matmul(wps[0:64, 0:64], wmm[:], wmm[:], start=True, stop=True)

            # v lhsT layout: col 0 = ones (softmax den lands in zt row 0),
            # cols 64..127 = v dims (z in zt rows 64..127 — PSUM partition
            # bases must be 32-aligned). Cols 1..63 are never read back but
            # must be finite for the matmul, so zero them.
            v_sb = cp.tile([128, NKT, HPC, 128], BF16, tag="v")
            nc.vector.memset(v_sb[:, :, :, 0:64], 0.0)
            nc.vector.memset(v_sb[:, :, :, 0:1], 1.0)

            # ---- v projection: v[s, c] (s on partitions), interleaved with
            # the q/k projection in x-DMA arrival order ----
            def emit_vproj(ts):
                for t in ts:
                    ps = ppool.tile([128, 512], F32, tag="proj", name=f"v{t}")
                    xt = x_t[t // 4]
                    c0 = (t % 4) * 128
                    for kb in range(KB):
                        nc.tensor.matmul(
                            ps[:, 0:256],
                            xt[:, kb, c0:c0 + 128],
                            wv[:, kb, :],
                            start=(kb == 0),
                            stop=(kb == KB - 1),
                        )
                    nc.vector.tensor_copy(
                        v_sb[:, t, :, 64:128],
                        ps[:, 0:256].rearrange("p (h d) -> p h d", h=HPC),
                    )

            # ---- q/k projection, transposed: [128 = 2 heads x 64, S] ----
            # mt 0,1 = q pairs; mt 2,3 = k pairs.
            T = [cp.tile([128, S], BF16, tag=f"T{mt}", name=f"T{mt}") for mt in range(4)]
            Tpre = [cp.tile([128, S], BF16, tag=f"Tpre{mt}", name=f"Tpre{mt}") for mt in range(4)]
            zsb = [cp.tile([128, S], BF16, tag=f"zsb{p}", name=f"zsb{p}") for p in range(2)]

            def emit_qk_chunk(n, mts, on_vector=False):
                # projection for one 512-col x chunk
                for mt in mts:
                    ps = ppool.tile([128, 512], F32, tag="proj", name=f"qk{mt}_{n}")
                    for kb in range(KB):
                        nc.tensor.matmul(
                            ps[:],
                            wqk[:, mt, kb, :],
                            x_t[n][:, kb],
                            start=(kb == 0),
                            stop=(kb == KB - 1),
                        )
                    dst = Tpre[mt][:, n * 512:(n + 1) * 512]
                    if on_vector:
                        nc.vector.tensor_copy(dst, ps[:])
                    else:
                        nc.scalar.copy(dst, ps[:])

            def emit_rope(nn, mts):
                # RoPE on one 1024-col half (partition-dim rotate-half via DMAs)
                lo = nn * 1024
                for mt in mts:
                    sw = work.tile([128, 1024], BF16, tag="sw")
                    for g in range(4):
                        src = (g + 1) if g % 2 == 0 else (g - 1)
                        # sync queue: a DMA issue occupies the issuing engine
                        # ~0.65us, and the scalar engine is the exp pacer
                        # during the attention groups where half-1 RoPE runs
                        nc.sync.dma_start(
                            sw[g * 32:(g + 1) * 32, :],
                            Tpre[mt][src * 32:(src + 1) * 32, lo:lo + 1024],
                        )
                    t1 = work.tile([128, 1024], BF16, tag="t1")
                    nc.vector.tensor_mul(t1[:], Tpre[mt][:, lo:lo + 1024], cos_sb[:, lo:lo + 1024])
                    t2 = work.tile([128, 1024], BF16, tag="t2")
                    nc.vector.tensor_mul(t2[:], sw[:], sin_sb[:, lo:lo + 1024])
                    nc.vector.tensor_add(T[mt][:, lo:lo + 1024], t1[:], t2[:])

            def emit_qk_proj_half(nn, mts=(0, 2, 1, 3), on_vector=False):
                for mt in mts:
                    for n2 in range(2):
                        emit_qk_chunk(nn * 2 + n2, (mt,), on_vector)
                    emit_rope(nn, (mt,))

            # minimal front: only what attention j=0 needs (v tiles 0-3 and
            # the RoPE'd half-0 q/k). v t4-15 stream into the attention
            # groups below, where the exp stream paces the PE and leaves
            # slack to fill.
            emit_vproj(range(0, 4))
            emit_qk_chunk(0, (0, 2))
            emit_qk_chunk(1, (0, 2))
            emit_rope(0, (0, 2))

            # ---- attention (scores^T, exp, attn@v with den row) ----
            # second-half projections interleave at pair granularity: one
            # ~7us PE burst per attention group keeps ACT's 2-deep score
            # pipeline from draining during a single long projection stretch
            # half-1 q/k projection schedule: every mt must land strictly
            # before the first group whose SCORES read it (j=2 groups read
            # their own pair's half-1 q AND k) — emission is post-exp, so
            # an mt emitted at (j,p) is only safe for groups after (j,p)
            proj1_at = {(0, 1): (2,), (1, 0): (3,), (1, 1): (0,), (2, 0): (1,)}
            vproj_at = {(0, 1): range(4, 8), (1, 1): range(8, 12), (2, 1): range(12, 16)}
            for j in range(NQC):
                for pair in range(2):
                    kmax = 4 * (j + 1)
                    iorder = list(range(kmax))
                    exs = {}
                    for i in iorder:
                        qlo = max(0, 128 * i - QC * j)
                        diag = 128 * i >= QC * j
                        sc = spool.tile([128, 2, QC], F32, tag="sc")
                        for hh in range(2):
                            nc.tensor.matmul(
                                sc[:, hh, qlo:QC],
                                T[2 + pair][hh * 64:(hh + 1) * 64, 128 * i:128 * (i + 1)],
                                T[pair][hh * 64:(hh + 1) * 64, QC * j + qlo:QC * (j + 1)],
                                start=True,
                                stop=not diag,
                                skip_group_check=diag,
                            )
                        if diag:
                            # causal mask on the PE: accumulate maskq[q,k]
                            # (-1e9 above diagonal) over the 128-wide strip
                            nc.tensor.matmul(
                                sc[:, :, qlo:qlo + 128],
                                maskq_sb[:],
                                idm_sb[:],
                                start=False,
                                stop=True,
                                skip_group_check=True,
                            )
                        ex = epool.tile([128, 2, QC], BF16, tag="ex", name=f"ex{i}")
                        nc.scalar.activation(
                            ex[:, :, qlo:], sc[:, :, qlo:], Exp, scale=0.125
                        )
                        exs[i] = (ex, qlo)
                    # deferred projection / out-proj work lands here: after
                    # the group's scores+exp stream (so exp starts ASAP) but
                    # in the window where ACT paces the PE
                    if j == 0 and pair == 0:
                        # q/k half-0 for the second head pair (used from j0p1)
                        emit_qk_chunk(0, (1, 3))
                        emit_qk_chunk(1, (1, 3))
                        emit_rope(0, (1, 3))
                    if (j, pair) in proj1_at:
                        emit_qk_proj_half(1, mts=proj1_at[(j, pair)], on_vector=True)
                    if (j, pair) in vproj_at:
                        emit_vproj(vproj_at[(j, pair)])
                    if j > 0:
                        # previous chunk's out-proj, half per group for
                        # finer-grained PE fill
                        _emit_outproj(nc, ppool, obp, wo, zsb, out_d, j - 1,
                                      ms=range(4) if pair == 0 else range(4, 8))
                    for hh in range(2):
                        zt = zpool.tile([128, QC], F32, tag="zt")
                        for n_i, i in enumerate(iorder):
                            ex, qlo = exs[i]
                            nc.tensor.matmul(
                                zt[:, qlo:],
                                v_sb[:, i, pair * 2 + hh, :],
                                ex[:, hh, qlo:],
                                start=(n_i == 0),
                                stop=(n_i == kmax - 1),
                            )
                        # normalize: z rows 64..127 by den row 0
                        # (copy den off PSUM first: the custom-DVE recip's
                        # PSUM read path races the PE accumulation)
                        den = nrm.tile([1, QC], F32, tag="den")
                        nc.vector.tensor_copy(den[:], zt[0:1, :])
                        rec = nrm.tile([1, QC], F32, tag="rec")
                        nc.vector.reciprocal_approx_fast(rec[:], den[:])
                        bc = nrm.tile([64, QC], F32, tag="bc")
                        nc.gpsimd.partition_broadcast(bc[:], rec[:])
                        nc.vector.tensor_mul(
                            zsb[pair][hh * 64:(hh + 1) * 64, QC * j:QC * (j + 1)],
                            zt[64:128, :],
                            bc[:],
                        )
            # final out-proj: attention pools are idle now — rotate po tiles
            # through 6 PSUM slots so the MMs aren't paced by the ob copies
            _emit_outproj(nc, ppool, obp, wo, zsb, out_d, NQC - 1,
                          psum_pools=[(ppool, "proj"), (spool, "sc"), (zpool, "zt")])

    nc.compile()
    return nc


def _emit_outproj(nc, ppool, obp, wo, zsb, out_d, j, pairs=(0, 1), pos=None, done=True,
                  psum_pools=None, ms=range(8)):
    pools = psum_pools or [(ppool, "proj")]
    for m in ms:
        pool, ptag = pools[m % len(pools)]
        po = pool.tile([128, 512], F32, tag=ptag,
                       name=f"po{j}_{m}") if pos is None else pos[m]
        for p in pairs:
            nc.tensor.matmul(
                po[:],
                wo[:, p, m * 128:(m + 1) * 128],
                zsb[p][:, QC * j:QC * (j + 1)],
                start=(p == pairs[0] and pos is None),
                stop=(p == pairs[-1] and done),
            )
        if done:
            # final chunk: ACT and gpsimd are idle — split copies and DMAs
            # across engines/queues. Mid-attention they'd steal time from
            # exp (ACT) and the norm broadcasts (gpsimd), so DVE/sync only.
            final = psum_pools is not None
            ob = obp.tile([128, 512], BF16, tag="ob", name=f"ob{j}_{m}")
            if final and m % 2 == 0:
                nc.scalar.copy(ob[:], po[:])
            else:
                nc.vector.tensor_copy(ob[:], po[:])
            dq = nc.gpsimd if final and m % 2 == 1 else nc.sync
            dq.dma_start(out_d[m * 128:(m + 1) * 128, QC * j:QC * (j + 1)], ob[:])


def _rope_tables():
    inv_freq = 1.0 / (ROPE_BASE ** (np.arange(0, DH, 2, dtype=np.float32) / DH))
    t = np.arange(S, dtype=np.float32)
    freqs = np.outer(t, inv_freq)            # [S, 32]
    cosT = np.cos(freqs).T                   # [32, S]
    sinT = np.sin(freqs).T
    cos128 = np.concatenate([cosT, cosT, cosT, cosT], axis=0)
    sin128 = np.concatenate([-sinT, sinT, -sinT, sinT], axis=0)
    return cos128.astype(ml_dtypes.bfloat16), sin128.astype(ml_dtypes.bfloat16)


def _prep_in_maps(x, w_qkv, w_o):
    cos128, sin128 = _rope_tables()
    kp, qc = np.meshgrid(np.arange(128), np.arange(128), indexing="ij")
    # maskq[q, k] = -1e9 where k > q (lhsT of the PE mask matmul)
    maskq = np.where(kp > qc, -1e9, 0.0).T.astype(np.float32)
    idm1 = np.eye(128, dtype=np.float32)                              # [p, q]
    idm = np.ascontiguousarray(np.stack([idm1, idm1], axis=1))        # [128, 2, 128]

    in_maps = []
    for c in range(NCORE):
        b, hb = c // 4, (c % 4) * HPC
        xb = np.ascontiguousarray(x[b].T)                        # [D, S]
        x_sb = xb.reshape(KB, 128, S).transpose(1, 0, 2)         # [128, KB, S]
        x_sb = x_sb.reshape(128, KB, 4, 512).transpose(0, 2, 1, 3)  # [128, 4, KB, 512]

        wqk = np.empty((128, 4, KB, 128), np.float32)
        for pair in range(2):
            qrows = w_qkv[(hb + 2 * pair) * DH:(hb + 2 * pair + 2) * DH, :]   # [128, D]
            krows = w_qkv[D + (hb + 2 * pair) * DH:D + (hb + 2 * pair + 2) * DH, :]
            wqk[:, pair] = qrows.T.reshape(KB, 128, 128).transpose(1, 0, 2)
            wqk[:, 2 + pair] = krows.T.reshape(KB, 128, 128).transpose(1, 0, 2)

        vrows = w_qkv[2 * D + hb * DH:2 * D + (hb + HPC) * DH, :]             # [256, D]
        wv = vrows.T.reshape(KB, 128, 256).transpose(1, 0, 2)                 # [128, KB, 256]

        wo_blk = w_o[:, hb * DH:hb * DH + 256]                                # [1024, 256]
        wo = wo_blk.T.reshape(2, 128, 1024).transpose(1, 0, 2)                # [128, 2, 1024]

        in_maps.append({
            "x": x_sb.astype(ml_dtypes.bfloat16),
            "wqk": wqk.astype(ml_dtypes.bfloat16),
            "wv": wv.astype(ml_dtypes.bfloat16),
            "wo": wo.astype(ml_dtypes.bfloat16),
            "cos": cos128,
            "sin": sin128,
            "maskq": maskq.astype(ml_dtypes.bfloat16),
            "idm": idm.astype(ml_dtypes.bfloat16),
        })
    return in_maps


def get_nc():
    if "nc" not in _cache:
        _cache["nc"] = _build()
    return _cache["nc"]


def run(x, w_qkv, w_o, **runkw):
    nc = get_nc()
    in_maps = _prep_in_maps(np.asarray(x), np.asarray(w_qkv), np.asarray(w_o))
    res = run_bass_kernel_spmd(nc, in_maps, core_ids=list(range(NCORE)), **runkw)
    out = np.zeros((B, S, D), np.float32)
    for c in range(NCORE):
        out[c // 4] += res.results[c]["out"].astype(np.float32).T
    return out, res


def kernel(x, w_qkv, w_o):
    out, _ = run(x, w_qkv, w_o)
    return out


# revision 50
# speedup vs baseline: 1.0612x; 1.0062x over previous
"""Trainium2 Bass kernel: RoPE causal attention block (B=2, S=2048, D=1024, H=16).

Sharding: (batch, head-group) across 8 cores — core c handles batch c//4 and
heads (c%4)*4 .. +4 (tensor-parallel qkv/o shards). Each core computes a
partial output projection over its 256 channels; host sums the 4 partials per
batch (the unshard step) and transposes back.

Device-side layout notes:
 - x is fed transposed (d on partitions) so it serves as both lhsT (v proj)
   and rhs (q/k proj) without on-chip transposes. x DMA is n-major (all 8
   k-blocks per 512-col chunk) so projection work starts ~3us in.
 - q/k are produced transposed [d_head, S] so scores come out as
   scores^T [k_pos, q_pos]; softmax denominators come from a ones column
   prepended to v (attn@v then yields den at PSUM row 0, z rows 1..64).
 - causal mask is applied on the PE: an extra accumulating matmul
   (maskq as lhsT, a head-broadcast identity as rhs) adds -1e9 above the
   diagonal, keeping DVE out of the scores->exp critical path.
 - exp/normalize: exp on ACT straight out of PSUM, reciprocal read directly
   from PSUM row 0 on DVE (1-instr approx), broadcast on GpSimd, multiplied
   during the PSUM->SBUF move on DVE.
 - ~4us of dummy matmuls at kernel start keep the PE HAM activity window
   busy during the input DMAs so the projection phase starts at 2.4 GHz.
All matmuls run in bf16 (1 cyc/row on PE vs 4 for f32).
"""

import sys

sys.path.insert(0, "/opt/trn_rl_repo")

import numpy as np
import ml_dtypes

import concourse.bass as bass  # noqa: F401
import concourse.tile as tile
from concourse import bacc, mybir
from concourse.bass_utils import run_bass_kernel_spmd

BF16 = mybir.dt.bfloat16
F32 = mybir.dt.float32

B, S, D = 2, 2048, 1024
H, DH = 16, 64
NCORE = 8
HPC = 4          # heads per core
KB = D // 128    # 8 k-blocks over the model dim
QC = 512         # q-chunk width
NQC = S // QC    # 4 q-chunks
NKT = S // 128   # 16 k-tiles
ROPE_BASE = 10000.0

_cache = {}


def _build():
    nc = bacc.Bacc("TRN2", target_bir_lowering=False, debug=False, num_devices=NCORE)

    x_d = nc.declare_dram_parameter("x", [128, 4, KB, 512], BF16, isOutput=False)
    wqk_d = nc.declare_dram_parameter("wqk", [128, 4, KB, 128], BF16, isOutput=False)
    wv_d = nc.declare_dram_parameter("wv", [128, KB, 256], BF16, isOutput=False)
    wo_d = nc.declare_dram_parameter("wo", [128, 2, 1024], BF16, isOutput=False)
    cos_d = nc.declare_dram_parameter("cos", [128, S], BF16, isOutput=False)
    sin_d = nc.declare_dram_parameter("sin", [128, S], BF16, isOutput=False)
    maskq_d = nc.declare_dram_parameter("maskq", [128, 128], BF16, isOutput=False)
    idm_d = nc.declare_dram_parameter("idm", [128, 2, 128], BF16, isOutput=False)
    out_d = nc.declare_dram_parameter("out", [D, S], BF16, isOutput=True)

    Exp = mybir.ActivationFunctionType.Exp

    with tile.TileContext(nc) as tc:
        with (
            tc.tile_pool(name="const", bufs=1) as cp,
            tc.tile_pool(name="ppool", bufs=2, space="PSUM") as ppool,
            tc.tile_pool(name="spool", bufs=2, space="PSUM") as spool,
            tc.tile_pool(name="zpool", bufs=2, space="PSUM") as zpool,
            tc.tile_pool(name="epool", bufs=18) as epool,
            tc.tile_pool(name="work", bufs=2) as work,
            tc.tile_pool(name="nrm", bufs=2) as nrm,
            tc.tile_pool(name="ob", bufs=8) as obp,
        ):
            # ---- input DMAs. The DMA engines round-robin across all live
            # transfers, so the j0-critical set (wv, wqk, x0, x1, cos, sin)
            # is issued immediately across the three rings, while everything
            # else (x2, x3, wo, maskq, idm) is gated behind x1's completion
            # via a 1-element touch op so it can't steal bandwidth early ----
            wv = cp.tile([128, KB, 256], BF16, tag="wv")
            nc.scalar.dma_start(wv[:], wv_d[:])
            # wqk mt-major: one 0.25MB DMA per head-group column, in the
            # order the q/k projection consumes them
            wqk = cp.tile([128, 4, KB, 128], BF16, tag="wqk")
            for mt in (0, 2, 1, 3):
                nc.scalar.dma_start(wqk[:, mt], wqk_d[:, mt])
            # x n-major as four separate tiles; the DRAM copy is laid out
            # n-major too, so each chunk half is one DMA with contiguous
            # 4KB-per-partition runs
            x_t = [cp.tile([128, KB, 512], BF16, tag=f"x{n}", name=f"x{n}") for n in range(4)]
            for n in range(2):
                nc.sync.dma_start(x_t[n][:, 0:4], x_d[:, n, 0:4])
                nc.gpsimd.dma_start(x_t[n][:, 4:8], x_d[:, n, 4:8])
            cos_sb = cp.tile([128, S], BF16, tag="cos")
            nc.sync.dma_start(cos_sb[:], cos_d[:])
            sin_sb = cp.tile([128, S], BF16, tag="sin")
            nc.gpsimd.dma_start(sin_sb[:], sin_d[:])

            # gate: a 1-elem copy reading x1 releases the non-critical DMAs
            maskq_sb = cp.tile([128, 128], BF16, tag="maskq")
            idm_sb = cp.tile([128, 2, 128], BF16, tag="idm")
            wo = cp.tile([128, 2, 1024], BF16, tag="wo")
            gate_src = x_t[1][0:1, 0, 0:1]
            nc.vector.tensor_copy(x_t[2][0:1, 0, 0:1], gate_src)
            nc.vector.tensor_copy(x_t[3][0:1, 0, 0:1], gate_src)
            nc.vector.tensor_copy(maskq_sb[0:1, 0:1], gate_src)
            nc.vector.tensor_copy(idm_sb[0:1, 0, 0:1], gate_src)
            nc.vector.tensor_copy(wo[0:1, 0, 0:1], gate_src)
            for n in range(2, 4):
                nc.sync.dma_start(x_t[n][:, 0:4], x_d[:, n, 0:4])
                nc.gpsimd.dma_start(x_t[n][:, 4:8], x_d[:, n, 4:8])
            nc.scalar.dma_start(maskq_sb[:], maskq_d[:])
            nc.scalar.dma_start(idm_sb[:], idm_d[:])
            nc.sync.dma_start(wo[:], wo_d[:])

            # tiny memsets first so the PE warm-up isn't gated behind the
            # big v_sb memset on the DVE queue
            wmm = work.tile([1, 64], BF16, tag="wmm")
            nc.vector.memset(wmm[:], 0.0)
            warm = work.tile([1, 8], F32, tag="warm")
            nc.vector.memset(warm[:], 0.0)
            # warm the ACT exp table while DMAs stream in
            nc.scalar.activation(warm[:], warm[:], Exp, scale=1.0)

            # ~2.5us of dummy matmuls: keep the PE HAM activity window busy
            # during the input DMAs so the proj phase starts warm (2.4 GHz)
            wps = ppool.tile([128, 512], F32, tag="proj", name="warmps")
            for _ in range(64):
                nc.tensor.matmul(wps[0:64, 0:64], wmm[:], wmm[:], start=True, stop=True)

            # v lhsT layout: col 0 = ones (softmax den lands in zt row 0),
            # cols 64..127 = v dims (z in zt rows 64..127 — PSUM partition
            # bases must be 32-aligned). Cols 1..63 are never read back but
            # must be finite for the matmul, so zero them.
            v_sb = cp.tile([128, NKT, HPC, 128], BF16, tag="v")
            nc.vector.memset(v_sb[:, :, :, 0:64], 0.0)
            nc.vector.memset(v_sb[:, :, :, 0:1], 1.0)

            # ---- v projection: v[s, c] (s on partitions), interleaved with
            # the q/k projection in x-DMA arrival order ----
            def emit_vproj(ts):
                for t in ts:
                    ps = ppool.tile([128, 512], F32, tag="proj", name=f"v{t}")
                    xt = x_t[t // 4]
                    c0 = (t % 4) * 128
                    for kb in range(KB):
                        nc.tensor.matmul(
                            ps[:, 0:256],
                            xt[:, kb, c0:c0 + 128],
                            wv[:, kb, :],
                            start=(kb == 0),
                            stop=(kb == KB - 1),
                        )
                    nc.vector.tensor_copy(
                        v_sb[:, t, :, 64:128],
                        ps[:, 0:256].rearrange("p (h d) -> p h d", h=HPC),
                    )

            # ---- q/k projection, transposed: [128 = 2 heads x 64, S] ----
            # mt 0,1 = q pairs; mt 2,3 = k pairs.
            T = [cp.tile([128, S], BF16, tag=f"T{mt}", name=f"T{mt}") for mt in range(4)]
            Tpre = [cp.tile([128, S], BF16, tag=f"Tpre{mt}", name=f"Tpre{mt}") for mt in range(4)]
            zsb = [cp.tile([128, S], BF16, tag=f"zsb{p}", name=f"zsb{p}") for p in range(2)]

            def emit_qk_chunk(n, mts, on_vector=False):
                # projection for one 512-col x chunk
                for mt in mts:
                    ps = ppool.tile([128, 512], F32, tag="proj", name=f"qk{mt}_{n}")
                    for kb in range(KB):
                        nc.tensor.matmul(
                            ps[:],
                            wqk[:, mt, kb, :],
                            x_t[n][:, kb],
                            start=(kb == 0),
                            stop=(kb == KB - 1),
                        )
                    dst = Tpre[mt][:, n * 512:(n + 1) * 512]
                    if on_vector:
                        nc.vector.tensor_copy(dst, ps[:])
                    else:
                        nc.scalar.copy(dst, ps[:])

            def emit_rope(nn, mts):
                # RoPE on one 1024-col half (partition-dim rotate-half via DMAs)
                lo = nn * 1024
                for mt in mts:
                    sw = work.tile([128, 1024], BF16, tag="sw")
                    for g in range(4):
                        src = (g + 1) if g % 2 == 0 else (g - 1)
                        # sync queue: a DMA issue occupies the issuing engine
                        # ~0.65us, and the scalar engine is the exp pacer
                        # during the attention groups where half-1 RoPE runs
                        nc.sync.dma_start(
                            sw[g * 32:(g + 1) * 32, :],
                            Tpre[mt][src * 32:(src + 1) * 32, lo:lo + 1024],
                        )
                    t1 = work.tile([128, 1024], BF16, tag="t1")
                    nc.vector.tensor_mul(t1[:], Tpre[mt][:, lo:lo + 1024], cos_sb[:, lo:lo + 1024])
                    t2 = work.tile([128, 1024], BF16, tag="t2")
                    nc.vector.tensor_mul(t2[:], sw[:], sin_sb[:, lo:lo + 1024])
                    nc.vector.tensor_add(T[mt][:, lo:lo + 1024], t1[:], t2[:])

            def emit_qk_proj_half(nn, mts=(0, 2, 1, 3), on_vector=False):
                for mt in mts:
                    for n2 in range(2):
                        emit_qk_chunk(nn * 2 + n2, (mt,), on_vector)
                    emit_rope(nn, (mt,))

            # minimal front: only what attention j=0 needs (v tiles 0-3 and
            # the RoPE'd half-0 q/k). v t4-15 stream into the attention
            # groups below, where the exp stream paces the PE and leaves
            # slack to fill.
            emit_vproj(range(0, 4))
            emit_qk_chunk(0, (0, 2))
            emit_qk_chunk(1, (0, 2))
            emit_rope(0, (0, 2))

            # ---- attention (scores^T, exp, attn@v with den row) ----
            # second-half projections interleave at pair granularity: one
            # ~7us PE burst per attention group keeps ACT's 2-deep score
            # pipeline from draining during a single long projection stretch
            # half-1 q/k projection schedule: every mt must land strictly
            # before the first group whose SCORES read it (j=2 groups read
            # their own pair's half-1 q AND k) — emission is post-exp, so
            # an mt emitted at (j,p) is only safe for groups after (j,p)
            proj1_at = {(0, 1): (2,), (1, 0): (3,), (1, 1): (0,), (2, 0): (1,)}
            vproj_at = {(0, 1): range(4, 8), (1, 1): range(8, 12), (2, 1): range(12, 16)}
            for j in range(NQC):
                for pair in range(2):
                    kmax = 4 * (j + 1)
                    iorder = list(range(kmax))
                    exs = {}
                    for i in iorder:
                        qlo = max(0, 128 * i - QC * j)
                        diag = 128 * i >= QC * j
                        sc = spool.tile([128, 2, QC], F32, tag="sc")
                        for hh in range(2):
                            nc.tensor.matmul(
                                sc[:, hh, qlo:QC],
                                T[2 + pair][hh * 64:(hh + 1) * 64, 128 * i:128 * (i + 1)],
                                T[pair][hh * 64:(hh + 1) * 64, QC * j + qlo:QC * (j + 1)],
                                start=True,
                                stop=not diag,
                                skip_group_check=diag,
                            )
                        if diag:
                            # causal mask on the PE: accumulate maskq[q,k]
                            # (-1e9 above diagonal) over the 128-wide strip
                            nc.tensor.matmul(
                                sc[:, :, qlo:qlo + 128],
                                maskq_sb[:],
                                idm_sb[:],
                                start=False,
                                stop=True,
                                skip_group_check=True,
                            )
                        ex = epool.tile([128, 2, QC], BF16, tag="ex", name=f"ex{i}")
                        nc.scalar.activation(
                            ex[:, :, qlo:], sc[:, :, qlo:], Exp, scale=0.125
                        )
                        exs[i] = (ex, qlo)
                    # deferred projection / out-proj work lands here: after
                    # the group's scores+exp stream (so exp starts ASAP) but
                    # in the window where ACT paces the PE
                    if j == 0 and pair == 0:
                        # q/k half-0 for the second head pair (used from j0p1)
                        emit_qk_chunk(0, (1, 3))
                        emit_qk_chunk(1, (1, 3))
                        emit_rope(0, (1, 3))
                    if (j, pair) in proj1_at:
                        emit_qk_proj_half(1, mts=proj1_at[(j, pair)], on_vector=True)
                    if (j, pair) in vproj_at:
                        emit_vproj(vproj_at[(j, pair)])
                    if j > 0:
                        # previous chunk's out-proj, half per group for
                        # finer-grained PE fill
                        _emit_outproj(nc, ppool, obp, wo, zsb, out_d, j - 1,
                                      ms=range(4) if pair == 0 else range(4, 8))
                    for hh in range(2):
                        zt = zpool.tile([128, QC], F32, tag="zt")
                        for n_i, i in enumerate(iorder):
                            ex, qlo = exs[i]
                            nc.tensor.matmul(
                                zt[:, qlo:],
                                v_sb[:, i, pair * 2 + hh, :],
                                ex[:, hh, qlo:],
                                start=(n_i == 0),
                                stop=(n_i == kmax - 1),
                            )
                        # normalize: z rows 64..127 by den row 0
                        # (copy den off PSUM first: the custom-DVE recip's
                        # PSUM read path races the PE accumulation)
                        den = nrm.tile([1, QC], F32, tag="den")
                        nc.vector.tensor_copy(den[:], zt[0:1, :])
                        rec = nrm.tile([1, QC], F32, tag="rec")
                        nc.vector.reciprocal_approx_fast(rec[:], den[:])
                        bc = nrm.tile([64, QC], F32, tag="bc")
                        nc.gpsimd.partition_broadcast(bc[:], rec[:])
                        nc.vector.tensor_mul(
                            zsb[pair][hh * 64:(hh + 1) * 64, QC * j:QC * (j + 1)],
                            zt[64:128, :],
                            bc[:],
                        )
            # final out-proj: attention pools are idle now — rotate po tiles
            # through 6 PSUM slots so the MMs aren't paced by the ob copies
            _emit_outproj(nc, ppool, obp, wo, zsb, out_d, NQC - 1,
                          psum_pools=[(ppool, "proj"), (spool, "sc"), (zpool, "zt")])

    nc.compile()
    return nc


def _emit_outproj(nc, ppool, obp, wo, zsb, out_d, j, pairs=(0, 1), pos=None, done=True,
                  psum_pools=None, ms=range(8)):
    pools = psum_pools or [(ppool, "proj")]
    for m in ms:
        pool, ptag = pools[m % len(pools)]
        po = pool.tile([128, 512], F32, tag=ptag,
                       name=f"po{j}_{m}") if pos is None else pos[m]
        for p in pairs:
            nc.tensor.matmul(
                po[:],
                wo[:, p, m * 128:(m + 1) * 128],
                zsb[p][:, QC * j:QC * (j + 1)],
                start=(p == pairs[0] and pos is None),
                stop=(p == pairs[-1] and done),
            )
        if done:
            # final chunk: ACT and gpsimd are idle — split copies and DMAs
            # across engines/queues. Mid-attention they'd steal time from
            # exp (ACT) and the norm broadcasts (gpsimd), so DVE/sync only.
            final = psum_pools is not None
            ob = obp.tile([128, 512], BF16, tag="ob", name=f"ob{j}_{m}")
            if final and m % 2 == 0:
                nc.scalar.copy(ob[:], po[:])
            else:
                nc.vector.tensor_copy(ob[:], po[:])
            dq = nc.gpsimd if final and m % 2 == 1 else nc.sync
            dq.dma_start(out_d[m * 128:(m + 1) * 128, QC * j:QC * (j + 1)], ob[:])


def _rope_tables():
    inv_freq = 1.0 / (ROPE_BASE ** (np.arange(0, DH, 2, dtype=np.float32) / DH))
    t = np.arange(S, dtype=np.float32)
    freqs = np.outer(t, inv_freq)            # [S, 32]
    cosT = np.cos(freqs).T                   # [32, S]
    sinT = np.sin(freqs).T
    cos128 = np.concatenate([cosT, cosT, cosT, cosT], axis=0)
    sin128 = np.concatenate([-sinT, sinT, -sinT, sinT], axis=0)
    return cos128.astype(ml_dtypes.bfloat16), sin128.astype(ml_dtypes.bfloat16)


def _prep_in_maps(x, w_qkv, w_o):
    cos128, sin128 = _rope_tables()
    kp, qc = np.meshgrid(np.arange(128), np.arange(128), indexing="ij")
    # maskq[q, k] = -1e9 where k > q (lhsT of the PE mask matmul)
    maskq = np.where(kp > qc, -1e9, 0.0).T.astype(np.float32)
    idm1 = np.eye(128, dtype=np.float32)                              # [p, q]
    idm = np.ascontiguousarray(np.stack([idm1, idm1], axis=1))        # [128, 2, 128]

    in_maps = []
    for c in range(NCORE):
        b, hb = c // 4, (c % 4) * HPC
        xb = np.ascontiguousarray(x[b].T)                        # [D, S]
        x_sb = xb.reshape(KB, 128, S).transpose(1, 0, 2)         # [128, KB, S]
        x_sb = x_sb.reshape(128, KB, 4, 512).transpose(0, 2, 1, 3)  # [128, 4, KB, 512]

        wqk = np.empty((128, 4, KB, 128), np.float32)
        for pair in range(2):
            qrows = w_qkv[(hb + 2 * pair) * DH:(hb + 2 * pair + 2) * DH, :]   # [128, D]
            krows = w_qkv[D + (hb + 2 * pair) * DH:D + (hb + 2 * pair + 2) * DH, :]
            wqk[:, pair] = qrows.T.reshape(KB, 128, 128).transpose(1, 0, 2)
            wqk[:, 2 + pair] = krows.T.reshape(KB, 128, 128).transpose(1, 0, 2)

        vrows = w_qkv[2 * D + hb * DH:2 * D + (hb + HPC) * DH, :]             # [256, D]
        wv = vrows.T.reshape(KB, 128, 256).transpose(1, 0, 2)                 # [128, KB, 256]

        wo_blk = w_o[:, hb * DH:hb * DH + 256]                                # [1024, 256]
        wo = wo_blk.T.reshape(2, 128, 1024).transpose(1, 0, 2)                # [128, 2, 1024]

        in_maps.append({
            "x": x_sb.astype(ml_dtypes.bfloat16),
            "wqk": wqk.astype(ml_dtypes.bfloat16),
            "wv": wv.astype(ml_dtypes.bfloat16),
            "wo": wo.astype(ml_dtypes.bfloat16),
            "cos": cos128,
            "sin": sin128,
            "maskq": maskq.astype(ml_dtypes.bfloat16),
            "idm": idm.astype(ml_dtypes.bfloat16),
        })
    return in_maps


def get_nc():
    if "nc" not in _cache:
        _cache["nc"] = _build()
    return _cache["nc"]


def run(x, w_qkv, w_o, **runkw):
    nc = get_nc()
    in_maps = _prep_in_maps(np.asarray(x), np.asarray(w_qkv), np.asarray(w_o))
    res = run_bass_kernel_spmd(nc, in_maps, core_ids=list(range(NCORE)), **runkw)
    out = np.zeros((B, S, D), np.float32)
    for c in range(NCORE):
        out[c // 4] += res.results[c]["out"].astype(np.float32).T
    return out, res


def kernel(x, w_qkv, w_o):
    out, _ = run(x, w_qkv, w_o)
    return out
